# revision 8
# baseline (speedup 1.0000x reference)
"""Deformable Conv2d (DCNv2) Trainium2 Bass kernel.

Sharding: 8 cores = (batch b in 0..3) x (image half in 0..1).
Pixel order row-major: p = yl*128 + x (yl = local row).

Stage 5 design: one 4-corner gather per (tap k, block of 8 rows) from an
HBM table x4 whose row q packs the 4 bilinear corners
[xcv[q], xcv[q+1], xcv[q+130], xcv[q+131]] (128 ch each, fp16, 1KB/row).
Gather is pixel-major (transpose=False): partition = x, group = row.
Bilinear weights applied via a stride-0-broadcast tensor_tensor from an
SBUF tile w4c[128x, (yl,k,corner)], corner-sum on DVE, transpose to
channel-major on TensorE, 9-tap accumulating matmuls into PSUM.
Gathers round-robin over 4 SWDGE queues.
"""
import numpy as np
from contextlib import ExitStack

import concourse.bass as bass
import concourse.tile as tile
from concourse import bacc, mybir
from concourse.bass_utils import run_bass_kernel_spmd

B, C, H, W = 4, 128, 128, 128
KK = 9
COUT = 128
N_CORES = 8
HHALF = 64
P = HHALF * W              # 8192
CVX = 130
CVY = 134
NCV = CVY * CVX            # 17420
F32 = mybir.dt.float32
F16 = mybir.dt.float16
I16 = mybir.dt.int16
I32 = mybir.dt.int32

_CACHE = {}


def _io(nc):
    aps = {}
    aps["x4"] = nc.dram_tensor("x4", [NCV, 512], F16, kind="ExternalInput").ap()
    aps["xcv"] = nc.dram_tensor("xcv", [128, NCV], F16, kind="ExternalInput").ap()
    aps["offw"] = nc.dram_tensor("offw", [128, KK * 27], F16, kind="ExternalInput").ap()
    aps["offb"] = nc.dram_tensor("offb", [128, 1], F32, kind="ExternalInput").ap()
    aps["wT"] = nc.dram_tensor("wT", [128, KK * 128], F16, kind="ExternalInput").ap()
    aps["byx"] = nc.dram_tensor("byx", [128, HHALF * 18], F32, kind="ExternalInput").ap()
    aps["ident"] = nc.dram_tensor("ident", [128, 128], F32, kind="ExternalInput").ap()
    aps["id16"] = nc.dram_tensor("id16", [128, 128], F16, kind="ExternalInput").ap()
    aps["cst"] = nc.dram_tensor("cst", [128, 3], F32, kind="ExternalInput").ap()
    aps["out"] = nc.dram_tensor("out", [128, P], F32, kind="ExternalOutput").ap()
    return aps


def _build():
    AOP = mybir.AluOpType
    ACT = mybir.ActivationFunctionType
    nc = bacc.Bacc("TRN2", target_bir_lowering=False, debug=False,
                   enable_asserts=False, num_devices=N_CORES,
                   num_swdge_queues=4)
    aps = _io(nc)

    with tile.TileContext(nc) as tc, ExitStack() as ctx:
        pp = ctx.enter_context(tc.tile_pool(name="pp", bufs=1))
        big = ctx.enter_context(tc.tile_pool(name="big", bufs=1))
        ppost = ctx.enter_context(tc.tile_pool(name="ppost", bufs=1))

        xcv = big.tile([128, NCV], F16)
        nc.sync.dma_start(xcv[:], aps["xcv"])
        offw = pp.tile([128, KK * 27], F16)
        nc.sync.dma_start(offw[:], aps["offw"])
        offb = pp.tile([128, 1], F32)
        nc.sync.dma_start(offb[:], aps["offb"])
        wT = pp.tile([128, KK * 128], F16)
        nc.sync.dma_start(wT[:], aps["wT"])
        byx = pp.tile([128, HHALF * 18], F32)
        nc.sync.dma_start(byx[:], aps["byx"])
        ident = pp.tile([128, 128], F32)
        nc.sync.dma_start(ident[:], aps["ident"])
        id16 = pp.tile([128, 128], F16)
        nc.sync.dma_start(id16[:], aps["id16"])
        cst = pp.tile([128, 3], F32)   # [:,0]=qoff, [:,1]=clo, [:,2]=chi
        nc.sync.dma_start(cst[:], aps["cst"])

        # ---- Stage 1+2: conv -> transpose -> offT [128(x), 64(yl), 27] ----
        offT = ppost.tile([128, HHALF * 27], F32)
        with tc.tile_pool(name="psA", bufs=2, space="PSUM") as psA, \
             tc.tile_pool(name="psB", bufs=2, space="PSUM") as psB, \
             tc.tile_pool(name="sA", bufs=2) as sA:
            for t in range(16):
                psc_t = psA.tile([27, 512], F32)
                for k in range(KK):
                    ky, kx = k // 3, k % 3
                    yl0 = 4 * t
                    off_elem = (yl0 + ky + 33) * CVX + kx
                    xcva = xcv[:]
                    rhs = bass.AP(xcva.tensor, xcva.offset + off_elem,
                                  [[xcva.ap[0][0], 128], [CVX, 4], [1, 128]])
                    nc.tensor.matmul(psc_t[:], offw[:, k * 27:(k + 1) * 27], rhs,
                                     start=(k == 0), stop=(k == KK - 1))
                ofsb = sA.tile([27, 512], F32, name="ofsb", tag="ofsb")
                nc.vector.tensor_scalar(ofsb[:], psc_t[:],
                                        offb[:27, :], None, op0=AOP.add)
                pst_t = psB.tile([128, 108], F32)
                for j2 in range(4):
                    nc.tensor.transpose(pst_t[:, j2 * 27:(j2 + 1) * 27],
                                        ofsb[:27, j2 * 128:(j2 + 1) * 128],
                                        ident[:27, :27])
                nc.vector.tensor_copy(offT[:, t * 108:(t + 1) * 108], pst_t[:])
        offT3 = offT[:].rearrange("p (y c) -> p y c", c=27)

        # ---- Stage 3: offset post-processing (layout [128x, yl(64), ...]) --
        _tn = [0]

        def t1152():
            _tn[0] += 1
            return ppost.tile([128, HHALF * 18], F32, name=f"t1152_{_tn[0]}",
                              tag=f"t1152_{_tn[0]}")

        def t576(dt=F32):
            _tn[0] += 1
            return ppost.tile([128, HHALF * 9], dt, name=f"t576_{_tn[0]}",
                              tag=f"t576_{_tn[0]}")

        pyx = t1152()
        nc.vector.tensor_scalar(pyx[:], offT3[:, :, 0:18], -32.0, 32.0,
                                op0=AOP.max, op1=AOP.min)
        nc.vector.tensor_tensor(pyx[:], pyx[:], byx[:], op=AOP.add)
        msk = t576()
        nc.scalar.activation(msk[:], offT3[:, :, 18:27], ACT.Sigmoid)
        fi = ppost.tile([128, HHALF * 18], I32)
        nc.vector.tensor_copy(fi[:], pyx[:])
        fl = t1152()
        nc.vector.tensor_copy(fl[:], fi[:])
        cmp = t1152()
        nc.vector.tensor_tensor(cmp[:], fl[:], pyx[:], op=AOP.is_gt)
        nc.vector.tensor_tensor(fl[:], fl[:], cmp[:], op=AOP.subtract)  # floor
        lyx = t1152()
        nc.vector.tensor_tensor(lyx[:], pyx[:], fl[:], op=AOP.subtract)
        omyx = t1152()
        nc.vector.tensor_scalar(omyx[:], lyx[:], -1.0, 1.0,
                                op0=AOP.mult, op1=AOP.add)

        v3 = lambda t: t[:].rearrange("p (y c) -> p y c", c=18)
        y_sl = lambda t: v3(t)[:, :, 0:9]
        x_sl = lambda t: v3(t)[:, :, 9:18]
        fly, flx = y_sl(fl), x_sl(fl)

        vtmp = ppost.tile([128, HHALF * 9], F32, name="vtmp", tag="vtmp")

        def vmask(src, lo, hi):
            a = t576()
            nc.vector.tensor_scalar(a[:], src, lo, None, op0=AOP.is_ge)
            nc.vector.tensor_scalar(vtmp[:], src, hi, None, op0=AOP.is_le)
            nc.vector.tensor_tensor(a[:], a[:], vtmp[:], op=AOP.mult)
            return a
        vy0 = vmask(fly, -0.5, 127.5)
        vy1 = vmask(fly, -1.5, 126.5)
        vx0 = vmask(flx, -1.5, 128.5)
        vx1 = vmask(flx, -1.5, 127.5)

        # clamped coords: y0c in [clo, chi]; x0c in [-1, 128]
        y0c, x0c = t576(), t576()
        nc.vector.tensor_scalar(y0c[:], fly, cst[:, 1:2], cst[:, 2:3],
                                op0=AOP.max, op1=AOP.min)
        nc.vector.tensor_scalar(x0c[:], flx, -1.0, 128.0, op0=AOP.max, op1=AOP.min)

        # weights
        A0, A1, B0, B1 = t576(), t576(), t576(), t576()
        nc.vector.tensor_tensor(A0[:], y_sl(omyx), vy0[:], op=AOP.mult)
        nc.vector.tensor_tensor(A0[:], A0[:], msk[:], op=AOP.mult)
        nc.vector.tensor_tensor(A1[:], y_sl(lyx), vy1[:], op=AOP.mult)
        nc.vector.tensor_tensor(A1[:], A1[:], msk[:], op=AOP.mult)
        nc.vector.tensor_tensor(B0[:], x_sl(omyx), vx0[:], op=AOP.mult)
        nc.vector.tensor_tensor(B1[:], x_sl(lyx), vx1[:], op=AOP.mult)

        # w4c[128x, (yl*9k + k)*4 + j] fp16, corner j = (r,s)
        w4c = ppost.tile([128, HHALF * KK * 4], F16)
        w4ca = w4c[:]
        for j, (Ar, Bs) in enumerate([(A0, B0), (A0, B1), (A1, B0), (A1, B1)]):
            nc.vector.tensor_tensor(vtmp[:], Ar[:], Bs[:], op=AOP.mult)
            dst = bass.AP(w4ca.tensor, w4ca.offset + j,
                          [[w4ca.ap[0][0], 128], [4, HHALF * KK]])
            nc.vector.tensor_copy(dst, vtmp[:])

        # gather indices q = y0c*130 + x0c + qoff  [128x, (yl,k)] f32
        qf = t576()
        nc.vector.scalar_tensor_tensor(qf[:], y0c[:], 130.0, x0c[:],
                                       op0=AOP.mult, op1=AOP.add)
        nc.vector.tensor_scalar(qf[:], qf[:], cst[:, 0:1], None, op0=AOP.add)

        # ---- Stage 4: on-chip wrap-16 idx build ----
        # idxw[p, k*512 + yl*8 + xhi] = q(yl, x = xhi*16 + p%16)
        qfa = qf[:]
        qS = ppost.tile([HHALF, KK * 128], F32)   # [64yl, (k, xlo16, xhi8)]
        idxw = ppost.tile([128, KK * 512], I16)
        with tc.tile_pool(name="psQ", bufs=3, space="PSUM") as psQ:
            for k in range(KK):
                psq = psQ.tile([HHALF, 128], F32, name="psq1", tag="psq1")
                src = bass.AP(qfa.tensor, qfa.offset + k,
                              [[qfa.ap[0][0], 128], [KK, HHALF]])
                nc.tensor.transpose(psq[:], src, ident[:, :])
                psqa = psq[:]
                srcv = bass.AP(psqa.tensor, psqa.offset,
                               [[psqa.ap[0][0], HHALF], [1, 16], [16, 8]])
                qSa = qS[:]
                dstv = bass.AP(qSa.tensor, qSa.offset + k * 128,
                               [[qSa.ap[0][0], HHALF], [8, 16], [1, 8]])
                nc.vector.tensor_copy(dstv, srcv)
            for k in range(KK):
                for xhi in range(8):
                    psq2 = psQ.tile([16, HHALF], F32, name="psq2", tag="psq2")
                    qSa = qS[:]
                    srcT = bass.AP(qSa.tensor, qSa.offset + k * 128 + xhi,
                                   [[qSa.ap[0][0], HHALF], [8, 16]])
                    nc.tensor.transpose(psq2[:], srcT, ident[:HHALF, :HHALF])
                    ia = idxw[:]
                    dst = bass.AP(ia.tensor, ia.offset + k * 512 + xhi,
                                  [[ia.ap[0][0], 16], [8, HHALF]])
                    nc.vector.tensor_copy(dst, psq2[:])
        # replicate wrap group to all 8 16-partition groups (doubling)
        for span in (16, 32, 64):
            nc.sync.dma_start(idxw[span:2 * span, :], idxw[0:span, :])

        # ---- Stage 5: gather + weight + corner-sum + transpose + matmul ----
        gp = ctx.enter_context(tc.tile_pool(name="gp", bufs=3))
        mp = ctx.enter_context(tc.tile_pool(name="mp", bufs=2))
        up = ctx.enter_context(tc.tile_pool(name="up", bufs=2))
        sp_ = ctx.enter_context(tc.tile_pool(name="sp", bufs=2))
        rp = ctx.enter_context(tc.tile_pool(name="rp", bufs=3))
        ps5 = ctx.enter_context(tc.tile_pool(name="ps5", bufs=3, space="PSUM"))
        pso = ctx.enter_context(tc.tile_pool(name="pso", bufs=2, space="PSUM"))
        outp = ctx.enter_context(tc.tile_pool(name="outp", bufs=2))

        x4a = aps["x4"]
        src4 = bass.AP(x4a.tensor, x4a.offset, [[512, NCV], [1, 512]])
        for c in range(8):
            pso_c = pso.tile([128, 1024], F32)
            for k in range(KK):
                gt = gp.tile([128, 8, 4, 128], F16)
                nc.gpsimd.dma_gather(
                    gt[:].rearrange("p a b c -> p a (b c)"), src4,
                    idxw[:, k * 512 + c * 64: k * 512 + (c + 1) * 64],
                    num_idxs=1024, num_idxs_reg=1024, elem_size=512,
                    elem_step=512, transpose=False, single_packet=True,
                    queue_num=(c * KK + k) % 4)
                m = mp.tile([128, 8, 4, 128], F16)
                w_b = bass.AP(w4ca.tensor,
                              w4ca.offset + (c * 8) * (KK * 4) + k * 4,
                              [[w4ca.ap[0][0], 128], [KK * 4, 8], [1, 4], [0, 128]])
                nc.vector.tensor_tensor(m[:], w_b, gt[:], op=AOP.mult)
                m2 = m[:].rearrange("p a b c -> p a (b c)")
                u = up.tile([128, 8, 256], F16)
                nc.vector.tensor_tensor(u[:], m2[:, :, 0:256], m2[:, :, 256:512],
                                        op=AOP.add)
                s = sp_.tile([128, 8, 128], F16)
                nc.vector.tensor_tensor(s[:], u[:, :, 0:128], u[:, :, 128:256],
                                        op=AOP.add)
                for g4 in range(2):
                    pst5 = ps5.tile([128, 512], F16)
                    for gg in range(4):
                        nc.tensor.transpose(pst5[:, gg * 128:(gg + 1) * 128],
                                            s[:, g4 * 4 + gg, :], id16[:, :])
                    rhs = rp.tile([128, 512], F16)
                    nc.scalar.activation(rhs[:], pst5[:], ACT.Copy)
                    nc.tensor.matmul(pso_c[:, g4 * 512:(g4 + 1) * 512],
                                     wT[:, k * 128:(k + 1) * 128], rhs[:],
                                     start=(k == 0), stop=(k == KK - 1))
            ot = outp.tile([128, 1024], F32)
            nc.vector.tensor_copy(ot[:], pso_c[:])
            nc.sync.dma_start(aps["out"][:, c * 1024:(c + 1) * 1024], ot[:])
    nc.compile()
    return nc


def _prep_inputs(x, offset_w, offset_b, weight):
    x = np.asarray(x, dtype=np.float32)
    offset_w = np.asarray(offset_w, dtype=np.float32)
    offset_b = np.asarray(offset_b, dtype=np.float32)
    weight = np.asarray(weight, dtype=np.float32)

    remap = np.array([2 * j for j in range(9)] +
                     [2 * j + 1 for j in range(9)] +
                     [18 + j for j in range(9)], dtype=np.int64)
    ow = offset_w[remap]
    ob = offset_b[remap]
    offw = np.zeros((128, KK * 27), dtype=np.float16)
    wT = np.zeros((128, KK * 128), dtype=np.float16)
    for k in range(KK):
        ky, kx = k // 3, k % 3
        offw[:, k * 27:(k + 1) * 27] = ow[:, :, ky, kx].T.astype(np.float16)
        wT[:, k * 128:(k + 1) * 128] = weight[:, :, ky, kx].T.astype(np.float16)
    offb = np.zeros((128, 1), dtype=np.float32)
    offb[:27, 0] = ob
    ident = np.eye(128, dtype=np.float32)
    id16 = np.eye(128, dtype=np.float16)

    in_maps = []
    for core in range(N_CORES):
        b, half = core // 2, core % 2
        r0 = half * HHALF
        xcv = np.zeros((128, CVY, CVX), dtype=np.float16)
        for t in range(CVY):
            gr = r0 + t - 34
            if 0 <= gr < H:
                xcv[:, t, 1:129] = x[b, :, gr, :].astype(np.float16)
        xflat = xcv.reshape(128, CVY * CVX)
        xq = np.ascontiguousarray(xflat.T)          # [NCV, 128]
        xpad = np.zeros((NCV + 132, 128), dtype=np.float16)
        xpad[:NCV] = xq
        x4 = np.concatenate([xpad[0:NCV], xpad[1:NCV + 1],
                             xpad[130:NCV + 130], xpad[131:NCV + 131]], axis=1)
        byx = np.zeros((128, HHALF, 18), dtype=np.float32)
        ylv = np.arange(HHALF)[None, :]
        xv = np.arange(128)[:, None]
        for k in range(KK):
            ky, kx = k // 3, k % 3
            byx[:, :, k] = r0 + ylv + ky - 1       # GLOBAL y base
            byx[:, :, 9 + k] = xv + kx - 1
        cst = np.zeros((128, 3), dtype=np.float32)
        cst[:, 0] = (34 - r0) * CVX + 1            # qoff
        cst[:, 1] = r0 - 34                        # clo
        cst[:, 2] = r0 + 98                        # chi
        in_maps.append({
            "x4": x4, "xcv": xflat,
            "offw": offw, "offb": offb, "wT": wT,
            "byx": byx.reshape(128, HHALF * 18),
            "ident": ident, "id16": id16, "cst": cst,
        })
    return in_maps


def kernel(x, offset_w, offset_b, weight):
    if "nc" not in _CACHE:
        _CACHE["nc"] = _build()
    nc = _CACHE["nc"]
    in_maps = _prep_inputs(x, offset_w, offset_b, weight)
    res = run_bass_kernel_spmd(nc, in_maps, list(range(N_CORES)))
    out = np.zeros((B, COUT, H, W), dtype=np.float32)
    for core in range(N_CORES):
        b, half = core // 2, core % 2
        r0 = half * HHALF
        o = res.results[core]["out"].reshape(COUT, HHALF, W)
        out[b, :, r0:r0 + HHALF, :] = o
    return out


def _build_null():
    """Same I/O as _build but no compute: for differential timing."""
    nc = bacc.Bacc("TRN2", target_bir_lowering=False, debug=False,
                   enable_asserts=False, num_devices=N_CORES)
    aps = _io(nc)
    with tile.TileContext(nc) as tc, ExitStack() as ctx:
        pool = ctx.enter_context(tc.tile_pool(name="sb", bufs=1))
        t = pool.tile([128, P], F32)
        nc.vector.memset(t[:], 0.0)
        nc.sync.dma_start(aps["out"], t[:])
    nc.compile()
    return nc
